# revision 12
# baseline (speedup 1.0000x reference)
"""AffinityPropagate prediction kernel for Trainium2 (8 NeuronCores).

Data-parallel over batch B=8: each core owns one image [480, 640].

Layout per core: 120 partitions x 4 image rows, rows padded to 642 cols.
All math runs in fp16 (DVE 2x mode). The center tap is folded into the
9-tap product tree with all weights pre-halved (w' = w/2), so the state
evolves as f * 2^-iter and stays in fp16 range with no per-iteration
rescale; one final ACT copy un-scales by 2^times.

State is kept in two aligned fp16 forms (double-buffered): fbs (image
cols at row offset 0, serving dc==0 taps) and fb (cols at offset 1,
serving dc==+-1 taps); every window is then 4B-aligned so every
tensor_tensor runs at DVE 2x. Per iteration DVE runs 11 instructions
(5 muls + 6 adds) with taps batched into multi-window access patterns
(tap-pair groups share one instruction) and the add tree batched over
concatenated product buffers. GPSIMD compute is deliberately unused in
the steady state: it shares an SBUF port with the DVE and measured 2.6x
slowdowns on concurrent tensor_tensor ops. The L3/final adds are split
{rows 0,3}/{rows 1,2} so halo-row DMAs fire one op early. DVE writes
fbs directly; ACT re-creates fb (shifted); SBUF-SBUF DMAs refresh
halos.
"""

import numpy as np
from contextlib import ExitStack

import concourse.bacc as bacc
import concourse.mybir as mybir
import concourse.tile as tile
from concourse import bass_utils
from concourse.ap import AP
from concourse.bass_interp import get_hw_module

B, CH, H, W = 8, 8, 480, 640
P = 120             # partitions (each holds ROWS_P rows)
ROWS_P = H // P     # 4
SLOTS = ROWS_P + 2  # + top/bottom halo row slots
WPAD = W + 2        # fb row: [0, img cols at 1..640, 0]
PX = ROWS_P * W     # 2560 compact px per partition

F32 = mybir.dt.float32
FP16 = mybir.dt.float16
I32 = mybir.dt.int32
AF = mybir.ActivationFunctionType
OP = mybir.AluOpType

# w9 slab order (taps grouped in instruction-pair order; center last):
#   0:(-1,-1) 1:(-1,+1) 2:(0,-1) 3:(0,+1) 4:(+1,-1) 5:(+1,+1)
#   6:(-1,0)  7:(+1,0)  8:center
# input affinity channel c (reference order) -> slab index:
SLAB_OF_CH = [0, 6, 1, 2, 3, 4, 7, 5]

# SCRATCH map (fp16 element offsets per partition)
AC16_O = 0            # [8, PX] fp16 copies of aff/2 in slab order (prep)
AST_O = 8 * PX        # 4-slot fp32 staging ring (2*PX units each)
SUMS_O = 16 * PX
SUMA_O = 17 * PX
REC16_O = 18 * PX
REC32_O = 19 * PX     # fp32 (2*PX units); also fp32 out staging at tail
ABS_O = 21 * PX       # fp16 abs scratch
SCR_UNITS = 22 * PX
RR_O = 0              # iter: two 3*PX-unit product rows
SCB = [6 * PX, 12 * PX]  # iter: X/D/E region, ring by parity

GPS_TREE = False      # gpsimd compute shares a DVE SBUF port: net loss (measured)


def _build(times: int):
    nc = bacc.Bacc("TRN2", debug=False, dynamic_dma_scratch_size=2048)
    aff_d = nc.dram_tensor("affinity", [CH, H * W], F32, kind="ExternalInput")
    feat_d = nc.dram_tensor("feature", [H, W], F32, kind="ExternalInput")
    out_d = nc.dram_tensor("out", [H, W], F32, kind="ExternalOutput")

    with tile.TileContext(nc) as tc, ExitStack() as ctx:
        pool = ctx.enter_context(tc.tile_pool(name="main", bufs=1))

        w9 = pool.tile([P, 9, PX], FP16)
        fbp = [pool.tile([P, SLOTS * WPAD], FP16, name=f"fb{i}")
               for i in range(2)]
        fbsp = [pool.tile([P, SLOTS * WPAD], FP16, name=f"fbs{i}")
                for i in range(2)]
        SCR = pool.tile([P, SCR_UNITS], FP16)

        scrf = SCR[:, :]

        def scr(o, n):
            return SCR[:, o:o + n]

        def scr32(o, n):
            return SCR[:, o:o + 2 * n].bitcast(F32)

        def mk(flat, off, dims):
            return AP(tensor=flat.tensor, offset=flat.offset + off,
                      ap=[list(flat.ap[0])] + [list(d) for d in dims])

        ac16 = scr(AC16_O, 8 * PX).rearrange("p (c x) -> p c x", x=PX)
        astage = [scr32(AST_O + 2 * PX * i, PX) for i in range(4)]
        abstmp = scr(ABS_O, PX)
        sums = scr(SUMS_O, PX)
        suma = scr(SUMA_O, PX)
        rec16 = scr(REC16_O, PX)
        sums32 = scr32(AST_O + 2 * PX, PX)   # staging slot 1, dead by then
        rec32 = scr32(REC32_O, PX)
        scrr = scr32(AST_O, PX)              # staging slot 0, dead by then

        def f3(t):
            return t[:, :].rearrange("p (s w) -> p s w", w=WPAD)

        fbv = [f3(t) for t in fbp]
        fbsv = [f3(t) for t in fbsp]

        # ---- input loads: every transfer is split into partition-halves
        # placed on two of the three DMA queues (gps SWDGE + the two hwdge
        # rings) so all queues finish together; items ordered so channels
        # arrive in sum-chain order, feature (not in the chain) last ----
        feat_v = feat_d[:, :].rearrange("(p r) w -> p r w", r=ROWS_P)
        aff_v = aff_d[:, :].rearrange("c (p x) -> c p x", x=PX)
        CHAIN = [0, 1, 6, 2, 3, 7, 4, 5]
        RING = [3, 0, 1, 2, 3, 0, 1, 2, 3]   # staging slot per load item
        QA = [nc.gpsimd, nc.sync, nc.scalar]
        PH = P // 2

        def item_trigs(i):
            slot = RING[i]
            src = feat_v if i == 8 else aff_v[CHAIN[i]]
            dst = astage[slot]
            if i == 8:
                dst = dst.rearrange("p (r w) -> p r w", w=W)
            QA[(2 * i) % 3].dma_start(dst[0:PH], src[0:PH])
            QA[(2 * i + 1) % 3].dma_start(dst[PH:P], src[PH:P])

        for i in range(3):
            item_trigs(i)

        # ---- zero-init padded state buffers (gpsimd; overlaps DMA) ----
        for t in fbp + fbsp:
            nc.gpsimd.memset(t[:, :], 0.0)

        # ---- fp16 convert + sum chain on DVE in channel-arrival order,
        # next item's triggers emitted as each staging slot frees ----
        for i, c in enumerate(CHAIN):
            st = astage[RING[i]]
            slab = SLAB_OF_CH[c]
            nc.vector.tensor_scalar(ac16[:, slab, :], st, 0.5, None, OP.mult)
            if i == 0:
                nc.vector.tensor_scalar(
                    sums.bitcast(I32), ac16[:, slab, :].bitcast(I32),
                    0x7FFF7FFF, None, OP.bitwise_and)
                nc.vector.tensor_copy(suma, ac16[:, slab, :])
            else:
                nc.vector.tensor_scalar(
                    abstmp.bitcast(I32), ac16[:, slab, :].bitcast(I32),
                    0x7FFF7FFF, None, OP.bitwise_and)
                nc.vector.tensor_add(sums, sums, abstmp)
                nc.vector.tensor_add(suma, suma, ac16[:, slab, :])
            if i + 3 <= 8:
                item_trigs(i + 3)

        # initial fp16 state: ACT is safe again here (all its queue
        # triggers are already emitted), overlapping the chain tail
        fst = astage[3].rearrange("p (r w) -> p r w", w=W)
        nc.scalar.activation(fbv[0][:, 1:5, 1:1 + W], fst, AF.Copy)
        nc.scalar.activation(fbsv[0][:, 1:5, 0:W], fst, AF.Copy)
        # initial halos on the gps queue (hwdge queues just drained)
        nc.gpsimd.dma_start(fbsv[0][1:P, 0, :], fbsv[0][0:P - 1, 4, :])
        nc.gpsimd.dma_start(fbsv[0][0:P - 1, 5, :], fbsv[0][1:P, 1, :])
        nc.gpsimd.dma_start(fbv[0][1:P, 0, :], fbv[0][0:P - 1, 4, :])
        nc.gpsimd.dma_start(fbv[0][0:P - 1, 5, :], fbv[0][1:P, 1, :])

        # ---- normalize: w9[0:8] = (a/2) * (1/S); center = 0.5 - suma/S ----
        nc.vector.tensor_copy(sums32, sums)
        nc.vector.reciprocal_approx_accurate(rec32, sums32, scrr)
        nc.vector.tensor_scalar(rec16, rec32, 0.5, None, OP.mult)
        # center weight on gpsimd, overlapping the big normalize mul on DVE
        ge = nc.gpsimd if GPS_TREE else nc.vector
        ge.tensor_mul(abstmp, suma, rec16)
        ge.tensor_scalar(w9[:, 8, :], abstmp, -1.0, 0.5, OP.mult, OP.add)
        nc.vector.tensor_mul(w9[:, 0:8, :], ac16[:, :, :],
                             rec16.unsqueeze(1).broadcast_to([P, 8, PX]))

        w9f = w9[:, :, :].rearrange("p c x -> p (c x)")
        out_v = out_d[:, :].rearrange("(p r) w -> p r w", r=ROWS_P)

        # ---- iterations ----
        for it in range(times):
            cb, nb = it % 2, (it + 1) % 2
            cfb, cfbs = fbp[cb][:, :], fbsp[cb][:, :]
            nfb3, nfbs3 = fbv[nb], fbsv[nb]
            nfbs = fbsp[nb][:, :]
            last = it == times - 1
            sc = SCB[it % 2]
            M4 = [[PX, 2], [W, ROWS_P], [1, W]]
            W4 = [[WPAD, ROWS_P], [1, W]]

            # muls, interior parts first (independent of halo DMAs so the
            # DVE never waits on the ~4.5us halo path at iter boundaries):
            # E (center) and D-interior read only DVE-written fbs rows.
            nc.vector.tensor_mul(
                scr(sc + 5 * PX, PX).rearrange("p (s w) -> p s w", w=W),
                mk(cfbs, WPAD, W4),
                w9[:, 8, :].rearrange("p (s w) -> p s w", w=W))
            nc.vector.tensor_mul(
                mk(scrf, sc + 3 * PX + W, [[3 * W, 2], [W, 3], [1, W]]),
                mk(cfbs, WPAD, [[WPAD, 2], [WPAD, 3], [1, W]]),
                mk(w9f, 6 * PX + W, [[3 * W, 2], [W, 3], [1, W]]))
            nc.vector.tensor_mul(mk(scrf, RR_O + 3 * PX, M4),
                                 mk(cfb, WPAD, [[2, 2]] + W4),
                                 mk(w9f, 2 * PX, M4))
            nc.vector.tensor_mul(mk(scrf, RR_O + W, [[PX, 2], [W, 3], [1, W]]),
                                 mk(cfb, WPAD, [[2, 2], [WPAD, 3], [1, W]]),
                                 mk(w9f, W, [[PX, 2], [W, 3], [1, W]]))
            nc.vector.tensor_mul(mk(scrf, RR_O + 2 * PX,
                                    [[3 * PX, 2], [W, 3], [1, W]]),
                                 mk(cfb, 2 * WPAD, [[2, 2], [WPAD, 3], [1, W]]),
                                 mk(w9f, 4 * PX, [[PX, 2], [W, 3], [1, W]]))
            # halo-row parts (1 row each; halos have landed by now)
            nc.vector.tensor_mul(
                mk(scrf, sc + 3 * PX, [[PX + 3 * W, 2], [1, W]]),
                mk(cfbs, 0, [[5 * WPAD, 2], [1, W]]),
                mk(w9f, 6 * PX, [[PX + 3 * W, 2], [1, W]]))
            nc.vector.tensor_mul(mk(scrf, RR_O, [[PX, 2], [1, W]]),
                                 mk(cfb, 0, [[2, 2], [1, W]]),
                                 mk(w9f, 0, [[PX, 2], [1, W]]))
            nc.vector.tensor_mul(mk(scrf, RR_O + 2 * PX + 3 * W,
                                    [[3 * PX, 2], [1, W]]),
                                 mk(cfb, 5 * WPAD, [[2, 2], [1, W]]),
                                 mk(w9f, 4 * PX + 3 * W, [[PX, 2], [1, W]]))
            # add tree over concatenated buffers
            nc.vector.tensor_add(scr(sc, 3 * PX), scr(RR_O, 3 * PX),
                                 scr(RR_O + 3 * PX, 3 * PX))
            l2 = mk(scrf, sc, [[2 * PX, 3], [1, PX]])
            nc.vector.tensor_tensor(l2, l2, mk(scrf, sc + PX,
                                               [[2 * PX, 3], [1, PX]]),
                                    OP.add)
            # L3/L4 unsplit: the interior/halo mul split gives halos a
            # ~10us window, so early-firing them no longer pays for ops
            nc.vector.tensor_add(scr(sc, PX), scr(sc, PX),
                                 scr(sc + 2 * PX, PX))
            nc.vector.tensor_tensor(
                mk(nfbs, WPAD, [[WPAD, ROWS_P], [1, W]]),
                mk(scrf, sc, [[W, ROWS_P], [1, W]]),
                mk(scrf, sc + 4 * PX, [[W, ROWS_P], [1, W]]), OP.add)
            if not last:
                nc.sync.dma_start(nfbs3[1:P, 0, :], nfbs3[0:P - 1, 4, :])
                nc.scalar.dma_start(nfbs3[0:P - 1, 5, :], nfbs3[1:P, 1, :])
            if not last:
                # rebuild fb (shifted copy) + its halos
                nc.scalar.activation(mk(fbp[nb][:, :], WPAD + 1,
                                        [[3 * WPAD, 2], [1, W]]),
                                     mk(nfbs, WPAD, [[3 * WPAD, 2], [1, W]]),
                                     AF.Copy)
                nc.scalar.dma_start(nfb3[1:P, 0, :], nfb3[0:P - 1, 4, :])
                nc.sync.dma_start(nfb3[0:P - 1, 5, :], nfb3[1:P, 1, :])
                nc.scalar.activation(mk(fbp[nb][:, :], 2 * WPAD + 1,
                                        [[WPAD, 2], [1, W]]),
                                     mk(nfbs, 2 * WPAD, [[WPAD, 2], [1, W]]),
                                     AF.Copy)
            else:
                # unscale to fp32 and store, {rows 0,3} right after L4a
                o32 = scr32(REC32_O, PX).rearrange("p (r w) -> p r w", w=W)
                sc16 = float(2.0 ** times)
                nc.scalar.activation(o32[:, 0::3, :], nfbs3[:, 1::3, 0:W],
                                     AF.Copy, scale=sc16)
                nc.sync.dma_start(out_v[:, 0::3, :], o32[:, 0::3, :])
                nc.scalar.activation(o32[:, 1:3, :], nfbs3[:, 2:4, 0:W],
                                     AF.Copy, scale=sc16)
                nc.scalar.dma_start(out_v[:, 1:3, :], o32[:, 1:3, :])

        if times == 0:
            nc.sync.dma_start(
                out_v, astage[3].rearrange("p (r w) -> p r w", w=W))

    nc.compile()
    nc.m = get_hw_module(nc.m)
    return nc


_CACHE = {}


def _get(times: int):
    if times not in _CACHE:
        _CACHE[times] = _build(times)
    return _CACHE[times]


def kernel(affinity, feature, times, _trace=False, _trace_kwargs=None):
    t = int(times)
    nc = _get(t)
    aff = np.ascontiguousarray(affinity, dtype=np.float32)
    fea = np.ascontiguousarray(feature, dtype=np.float32)
    in_maps = [
        {"affinity": aff[b].reshape(CH, H * W), "feature": fea[b, 0]}
        for b in range(B)
    ]
    res = bass_utils.run_bass_kernel_spmd(
        nc, in_maps, core_ids=list(range(B)),
        trace=_trace, **(_trace_kwargs or {}),
    )
    out = np.stack([res.results[b]["out"] for b in range(B)])[:, None]
    if _trace:
        return out.astype(np.float32), res
    return out.astype(np.float32)


# revision 13
# speedup vs baseline: 1.1948x; 1.1948x over previous
"""AffinityPropagate prediction kernel for Trainium2 (8 NeuronCores).

Data-parallel over batch B=8: each core owns one image [480, 640].

Layout per core: 120 partitions x 4 image rows, rows padded to 642 cols.
All math runs in fp16 (DVE 2x mode). The center tap is folded into the
9-tap product tree with all weights pre-halved (w' = w/2), so the state
evolves as f * 2^-iter and stays in fp16 range with no per-iteration
rescale; one final ACT copy un-scales by 2^times.

State is kept in two aligned fp16 forms (double-buffered): fbs (image
cols at row offset 0, serving dc==0 taps) and fb (cols at offset 1,
serving dc==+-1 taps); every window is then 4B-aligned so every
tensor_tensor runs at DVE 2x. Per iteration DVE runs 11 instructions
(5 muls + 6 adds) with taps batched into multi-window access patterns
(tap-pair groups share one instruction) and the add tree batched over
concatenated product buffers. GPSIMD compute is deliberately unused in
the steady state: it shares an SBUF port with the DVE and measured 2.6x
slowdowns on concurrent tensor_tensor ops. The L3/final adds are split
{rows 0,3}/{rows 1,2} so halo-row DMAs fire one op early. DVE writes
fbs directly; ACT re-creates fb (shifted); SBUF-SBUF DMAs refresh
halos.
"""

import numpy as np
from contextlib import ExitStack

import concourse.bacc as bacc
import concourse.mybir as mybir
import concourse.tile as tile
from concourse import bass_utils
from concourse.ap import AP
from concourse.bass_interp import get_hw_module

B, CH, H, W = 8, 8, 480, 640
P = 120             # partitions (each holds ROWS_P rows)
ROWS_P = H // P     # 4
SLOTS = ROWS_P + 2  # + top/bottom halo row slots
WPAD = W + 2        # fb row: [0, img cols at 1..640, 0]
PX = ROWS_P * W     # 2560 compact px per partition

F32 = mybir.dt.float32
FP16 = mybir.dt.float16
I32 = mybir.dt.int32
AF = mybir.ActivationFunctionType
OP = mybir.AluOpType

# w9 slab order (taps grouped in instruction-pair order; center last):
#   0:(-1,-1) 1:(-1,+1) 2:(0,-1) 3:(0,+1) 4:(+1,-1) 5:(+1,+1)
#   6:(-1,0)  7:(+1,0)  8:center
# input affinity channel c (reference order) -> slab index:
SLAB_OF_CH = [0, 6, 1, 2, 3, 4, 7, 5]

# SCRATCH map (fp16 element offsets per partition)
AC16_O = 0            # [8, PX] fp16 copies of aff/2 in slab order (prep)
AST_O = 8 * PX        # 4-slot fp32 staging ring (2*PX units each)
SUMS_O = 16 * PX
SUMA_O = 17 * PX
REC16_O = 18 * PX
REC32_O = 19 * PX     # fp32 (2*PX units); also fp32 out staging at tail
ABS_O = 21 * PX       # fp16 abs scratch
SCR_UNITS = 22 * PX
RR_O = 0              # iter: two 3*PX-unit product rows
SCB = [6 * PX, 12 * PX]  # iter: X/D/E region, ring by parity

GPS_TREE = False      # gpsimd compute shares a DVE SBUF port: net loss (measured)


def _build(times: int):
    nc = bacc.Bacc("TRN2", debug=False, dynamic_dma_scratch_size=2048)
    aff_d = nc.dram_tensor("affinity", [CH, H * W], F32, kind="ExternalInput")
    feat_d = nc.dram_tensor("feature", [H, W], F32, kind="ExternalInput")
    out_d = nc.dram_tensor("out", [H, W], F32, kind="ExternalOutput")

    with tile.TileContext(nc) as tc, ExitStack() as ctx:
        pool = ctx.enter_context(tc.tile_pool(name="main", bufs=1))

        w9 = pool.tile([P, 9, PX], FP16)
        fbp = [pool.tile([P, SLOTS * WPAD], FP16, name=f"fb{i}")
               for i in range(2)]
        fbsp = [pool.tile([P, SLOTS * WPAD], FP16, name=f"fbs{i}")
                for i in range(2)]
        SCR = pool.tile([P, SCR_UNITS], FP16)

        scrf = SCR[:, :]

        def scr(o, n):
            return SCR[:, o:o + n]

        def scr32(o, n):
            return SCR[:, o:o + 2 * n].bitcast(F32)

        def mk(flat, off, dims):
            return AP(tensor=flat.tensor, offset=flat.offset + off,
                      ap=[list(flat.ap[0])] + [list(d) for d in dims])

        ac16 = scr(AC16_O, 8 * PX).rearrange("p (c x) -> p c x", x=PX)
        astage = [scr32(AST_O + 2 * PX * i, PX) for i in range(4)]
        abstmp = scr(ABS_O, PX)
        sums = scr(SUMS_O, PX)
        suma = scr(SUMA_O, PX)
        rec16 = scr(REC16_O, PX)
        sums32 = scr32(AST_O + 2 * PX, PX)   # staging slot 1, dead by then
        rec32 = scr32(REC32_O, PX)
        scrr = scr32(AST_O, PX)              # staging slot 0, dead by then

        def f3(t):
            return t[:, :].rearrange("p (s w) -> p s w", w=WPAD)

        fbv = [f3(t) for t in fbp]
        fbsv = [f3(t) for t in fbsp]

        # ---- input loads: every transfer is split into partition-halves
        # placed on two of the three DMA queues (gps SWDGE + the two hwdge
        # rings) so all queues finish together; items ordered so channels
        # arrive in sum-chain order, feature (not in the chain) last ----
        feat_v = feat_d[:, :].rearrange("(p r) w -> p r w", r=ROWS_P)
        aff_v = aff_d[:, :].rearrange("c (p x) -> c p x", x=PX)
        CHAIN = [0, 1, 6, 2, 3, 7, 4, 5]
        RING = [3, 0, 1, 2, 3, 0, 1, 2, 3]   # staging slot per load item
        QA = [nc.gpsimd, nc.sync, nc.scalar]
        PH = P // 2

        def item_trigs(i):
            slot = RING[i]
            src = feat_v if i == 8 else aff_v[CHAIN[i]]
            dst = astage[slot]
            if i == 8:
                dst = dst.rearrange("p (r w) -> p r w", w=W)
            QA[(2 * i) % 3].dma_start(dst[0:PH], src[0:PH])
            QA[(2 * i + 1) % 3].dma_start(dst[PH:P], src[PH:P])

        for i in range(3):
            item_trigs(i)

        # ---- zero-init padded state buffers (gpsimd; overlaps DMA) ----
        for t in fbp + fbsp:
            nc.gpsimd.memset(t[:, :], 0.0)

        # ---- fp16 convert + sum chain on DVE in channel-arrival order,
        # next item's triggers emitted as each staging slot frees ----
        for i, c in enumerate(CHAIN):
            st = astage[RING[i]]
            slab = SLAB_OF_CH[c]
            nc.vector.tensor_scalar(ac16[:, slab, :], st, 0.5, None, OP.mult)
            if i == 0:
                nc.vector.tensor_scalar(
                    sums.bitcast(I32), ac16[:, slab, :].bitcast(I32),
                    0x7FFF7FFF, None, OP.bitwise_and)
                nc.vector.tensor_copy(suma, ac16[:, slab, :])
            else:
                nc.vector.tensor_scalar(
                    abstmp.bitcast(I32), ac16[:, slab, :].bitcast(I32),
                    0x7FFF7FFF, None, OP.bitwise_and)
                nc.vector.tensor_add(sums, sums, abstmp)
                nc.vector.tensor_add(suma, suma, ac16[:, slab, :])
            if i + 3 <= 8:
                item_trigs(i + 3)

        # initial fp16 state: ACT is safe again here (all its queue
        # triggers are already emitted), overlapping the chain tail
        fst = astage[3].rearrange("p (r w) -> p r w", w=W)
        nc.scalar.activation(fbv[0][:, 1:5, 1:1 + W], fst, AF.Copy)
        nc.scalar.activation(fbsv[0][:, 1:5, 0:W], fst, AF.Copy)
        # initial halos on the gps queue (hwdge queues just drained)
        nc.gpsimd.dma_start(fbsv[0][1:P, 0, :], fbsv[0][0:P - 1, 4, :])
        nc.gpsimd.dma_start(fbsv[0][0:P - 1, 5, :], fbsv[0][1:P, 1, :])
        nc.gpsimd.dma_start(fbv[0][1:P, 0, :], fbv[0][0:P - 1, 4, :])
        nc.gpsimd.dma_start(fbv[0][0:P - 1, 5, :], fbv[0][1:P, 1, :])

        # ---- normalize: w9[0:8] = (a/2) * (1/S); center = 0.5 - suma/S ----
        nc.vector.tensor_copy(sums32, sums)
        nc.vector.reciprocal_approx_accurate(rec32, sums32, scrr)
        nc.vector.tensor_scalar(rec16, rec32, 0.5, None, OP.mult)
        # center weight on gpsimd, overlapping the big normalize mul on DVE
        ge = nc.gpsimd if GPS_TREE else nc.vector
        ge.tensor_mul(abstmp, suma, rec16)
        ge.tensor_scalar(w9[:, 8, :], abstmp, -1.0, 0.5, OP.mult, OP.add)
        nc.vector.tensor_mul(w9[:, 0:8, :], ac16[:, :, :],
                             rec16.unsqueeze(1).broadcast_to([P, 8, PX]))

        w9f = w9[:, :, :].rearrange("p c x -> p (c x)")
        out_v = out_d[:, :].rearrange("(p r) w -> p r w", r=ROWS_P)

        # ---- iterations ----
        for it in range(times):
            cb, nb = it % 2, (it + 1) % 2
            cfb, cfbs = fbp[cb][:, :], fbsp[cb][:, :]
            nfb3, nfbs3 = fbv[nb], fbsv[nb]
            nfbs = fbsp[nb][:, :]
            last = it == times - 1
            sc = SCB[it % 2]
            M4 = [[PX, 2], [W, ROWS_P], [1, W]]
            W4 = [[WPAD, ROWS_P], [1, W]]

            # muls, interior parts first (independent of halo DMAs so the
            # DVE never waits on the ~4.5us halo path at iter boundaries):
            # E (center) and D-interior read only DVE-written fbs rows.
            nc.vector.tensor_mul(
                scr(sc + 5 * PX, PX).rearrange("p (s w) -> p s w", w=W),
                mk(cfbs, WPAD, W4),
                w9[:, 8, :].rearrange("p (s w) -> p s w", w=W))
            nc.vector.tensor_mul(
                mk(scrf, sc + 3 * PX + W, [[3 * W, 2], [W, 3], [1, W]]),
                mk(cfbs, WPAD, [[WPAD, 2], [WPAD, 3], [1, W]]),
                mk(w9f, 6 * PX + W, [[3 * W, 2], [W, 3], [1, W]]))
            nc.vector.tensor_mul(mk(scrf, RR_O + 3 * PX, M4),
                                 mk(cfb, WPAD, [[2, 2]] + W4),
                                 mk(w9f, 2 * PX, M4))
            nc.vector.tensor_mul(mk(scrf, RR_O + W, [[PX, 2], [W, 3], [1, W]]),
                                 mk(cfb, WPAD, [[2, 2], [WPAD, 3], [1, W]]),
                                 mk(w9f, W, [[PX, 2], [W, 3], [1, W]]))
            nc.vector.tensor_mul(mk(scrf, RR_O + 2 * PX,
                                    [[3 * PX, 2], [W, 3], [1, W]]),
                                 mk(cfb, 2 * WPAD, [[2, 2], [WPAD, 3], [1, W]]),
                                 mk(w9f, 4 * PX, [[PX, 2], [W, 3], [1, W]]))
            # halo-row parts (1 row each; halos have landed by now)
            nc.vector.tensor_mul(
                mk(scrf, sc + 3 * PX, [[PX + 3 * W, 2], [1, W]]),
                mk(cfbs, 0, [[5 * WPAD, 2], [1, W]]),
                mk(w9f, 6 * PX, [[PX + 3 * W, 2], [1, W]]))
            nc.vector.tensor_mul(mk(scrf, RR_O, [[PX, 2], [1, W]]),
                                 mk(cfb, 0, [[2, 2], [1, W]]),
                                 mk(w9f, 0, [[PX, 2], [1, W]]))
            nc.vector.tensor_mul(mk(scrf, RR_O + 2 * PX + 3 * W,
                                    [[3 * PX, 2], [1, W]]),
                                 mk(cfb, 5 * WPAD, [[2, 2], [1, W]]),
                                 mk(w9f, 4 * PX + 3 * W, [[PX, 2], [1, W]]))
            # add tree over concatenated buffers
            nc.vector.tensor_add(scr(sc, 3 * PX), scr(RR_O, 3 * PX),
                                 scr(RR_O + 3 * PX, 3 * PX))
            l2 = mk(scrf, sc, [[2 * PX, 3], [1, PX]])
            nc.vector.tensor_tensor(l2, l2, mk(scrf, sc + PX,
                                               [[2 * PX, 3], [1, PX]]),
                                    OP.add)
            # L3/L4 split {rows 0,3} then {rows 1,2}; halos fire after L4a
            nc.vector.tensor_tensor(
                mk(scrf, sc, [[3 * W, 2], [1, W]]),
                mk(scrf, sc, [[3 * W, 2], [1, W]]),
                mk(scrf, sc + 2 * PX, [[3 * W, 2], [1, W]]), OP.add)
            nc.vector.tensor_tensor(
                mk(nfbs, WPAD, [[3 * WPAD, 2], [1, W]]),
                mk(scrf, sc, [[3 * W, 2], [1, W]]),
                mk(scrf, sc + 4 * PX, [[3 * W, 2], [1, W]]), OP.add)
            if not last:
                nc.sync.dma_start(nfbs3[1:P, 0, :], nfbs3[0:P - 1, 4, :])
                nc.scalar.dma_start(nfbs3[0:P - 1, 5, :], nfbs3[1:P, 1, :])
            nc.vector.tensor_tensor(
                mk(scrf, sc + W, [[W, 2], [1, W]]),
                mk(scrf, sc + W, [[W, 2], [1, W]]),
                mk(scrf, sc + 2 * PX + W, [[W, 2], [1, W]]), OP.add)
            nc.vector.tensor_tensor(
                mk(nfbs, 2 * WPAD, [[WPAD, 2], [1, W]]),
                mk(scrf, sc + W, [[W, 2], [1, W]]),
                mk(scrf, sc + 4 * PX + W, [[W, 2], [1, W]]), OP.add)
            if not last:
                # rebuild fb (shifted copy) + its halos
                nc.scalar.activation(mk(fbp[nb][:, :], WPAD + 1,
                                        [[3 * WPAD, 2], [1, W]]),
                                     mk(nfbs, WPAD, [[3 * WPAD, 2], [1, W]]),
                                     AF.Copy)
                nc.scalar.dma_start(nfb3[1:P, 0, :], nfb3[0:P - 1, 4, :])
                nc.sync.dma_start(nfb3[0:P - 1, 5, :], nfb3[1:P, 1, :])
                nc.scalar.activation(mk(fbp[nb][:, :], 2 * WPAD + 1,
                                        [[WPAD, 2], [1, W]]),
                                     mk(nfbs, 2 * WPAD, [[WPAD, 2], [1, W]]),
                                     AF.Copy)
            else:
                # unscale to fp32 and store, {rows 0,3} right after L4a
                o32 = scr32(REC32_O, PX).rearrange("p (r w) -> p r w", w=W)
                sc16 = float(2.0 ** times)
                nc.scalar.activation(o32[:, 0::3, :], nfbs3[:, 1::3, 0:W],
                                     AF.Copy, scale=sc16)
                nc.sync.dma_start(out_v[:, 0::3, :], o32[:, 0::3, :])
                nc.scalar.activation(o32[:, 1:3, :], nfbs3[:, 2:4, 0:W],
                                     AF.Copy, scale=sc16)
                nc.scalar.dma_start(out_v[:, 1:3, :], o32[:, 1:3, :])

        if times == 0:
            nc.sync.dma_start(
                out_v, astage[3].rearrange("p (r w) -> p r w", w=W))

    nc.compile()
    nc.m = get_hw_module(nc.m)
    return nc


_CACHE = {}


def _get(times: int):
    if times not in _CACHE:
        _CACHE[times] = _build(times)
    return _CACHE[times]


def kernel(affinity, feature, times, _trace=False, _trace_kwargs=None):
    t = int(times)
    nc = _get(t)
    aff = np.ascontiguousarray(affinity, dtype=np.float32)
    fea = np.ascontiguousarray(feature, dtype=np.float32)
    in_maps = [
        {"affinity": aff[b].reshape(CH, H * W), "feature": fea[b, 0]}
        for b in range(B)
    ]
    res = bass_utils.run_bass_kernel_spmd(
        nc, in_maps, core_ids=list(range(B)),
        trace=_trace, **(_trace_kwargs or {}),
    )
    out = np.stack([res.results[b]["out"] for b in range(B)])[:, None]
    if _trace:
        return out.astype(np.float32), res
    return out.astype(np.float32)


# revision 14
# speedup vs baseline: 1.1998x; 1.0042x over previous
"""AffinityPropagate prediction kernel for Trainium2 (8 NeuronCores).

Data-parallel over batch B=8: each core owns one image [480, 640].

Layout per core: 120 partitions x 4 image rows, rows padded to 642 cols.
All math runs in fp16 (DVE 2x mode). The center tap is folded into the
9-tap product tree with all weights pre-halved (w' = w/2), so the state
evolves as f * 2^-iter and stays in fp16 range with no per-iteration
rescale; one final ACT copy un-scales by 2^times.

State is kept in two aligned fp16 forms (double-buffered): fbs (image
cols at row offset 0, serving dc==0 taps) and fb (cols at offset 1,
serving dc==+-1 taps); every window is then 4B-aligned so every
tensor_tensor runs at DVE 2x. Per iteration DVE runs 11 instructions
(5 muls + 6 adds) with taps batched into multi-window access patterns
(tap-pair groups share one instruction) and the add tree batched over
concatenated product buffers. GPSIMD compute is deliberately unused in
the steady state: it shares an SBUF port with the DVE and measured 2.6x
slowdowns on concurrent tensor_tensor ops. The L3/final adds are split
{rows 0,3}/{rows 1,2} so halo-row DMAs fire one op early. DVE writes
fbs directly; ACT re-creates fb (shifted); SBUF-SBUF DMAs refresh
halos.
"""

import numpy as np
from contextlib import ExitStack

import concourse.bacc as bacc
import concourse.mybir as mybir
import concourse.tile as tile
from concourse import bass_utils
from concourse.ap import AP
from concourse.bass_interp import get_hw_module

B, CH, H, W = 8, 8, 480, 640
P = 120             # partitions (each holds ROWS_P rows)
ROWS_P = H // P     # 4
SLOTS = ROWS_P + 2  # + top/bottom halo row slots
WPAD = W + 2        # fb row: [0, img cols at 1..640, 0]
PX = ROWS_P * W     # 2560 compact px per partition

F32 = mybir.dt.float32
FP16 = mybir.dt.float16
I32 = mybir.dt.int32
AF = mybir.ActivationFunctionType
OP = mybir.AluOpType

# w9 slab order (taps grouped in instruction-pair order; center last):
#   0:(-1,-1) 1:(-1,+1) 2:(0,-1) 3:(0,+1) 4:(+1,-1) 5:(+1,+1)
#   6:(-1,0)  7:(+1,0)  8:center
# input affinity channel c (reference order) -> slab index:
SLAB_OF_CH = [0, 6, 1, 2, 3, 4, 7, 5]

# SCRATCH map (fp16 element offsets per partition)
AC16_O = 0            # [8, PX] fp16 copies of aff/2 in slab order (prep)
AST_O = 8 * PX        # 4-slot fp32 staging ring (2*PX units each)
SUMS_O = 16 * PX
SUMA_O = 17 * PX
REC16_O = 18 * PX
REC32_O = 19 * PX     # fp32 (2*PX units); also fp32 out staging at tail
ABS_O = 21 * PX       # fp16 abs scratch
SCR_UNITS = 22 * PX
RR_O = 0              # iter: two 3*PX-unit product rows
SCB = [6 * PX, 12 * PX]  # iter: X/D/E region, ring by parity

GPS_TREE = False      # gpsimd compute shares a DVE SBUF port: net loss (measured)


def _build(times: int):
    nc = bacc.Bacc("TRN2", debug=False, dynamic_dma_scratch_size=2048)
    aff_d = nc.dram_tensor("affinity", [CH, H * W], F32, kind="ExternalInput")
    feat_d = nc.dram_tensor("feature", [H, W], F32, kind="ExternalInput")
    out_d = nc.dram_tensor("out", [H, W], F32, kind="ExternalOutput")

    with tile.TileContext(nc) as tc, ExitStack() as ctx:
        pool = ctx.enter_context(tc.tile_pool(name="main", bufs=1))

        w9 = pool.tile([P, 9, PX], FP16)
        fbp = [pool.tile([P, SLOTS * WPAD], FP16, name=f"fb{i}")
               for i in range(2)]
        fbsp = [pool.tile([P, SLOTS * WPAD], FP16, name=f"fbs{i}")
                for i in range(2)]
        SCR = pool.tile([P, SCR_UNITS], FP16)

        scrf = SCR[:, :]

        def scr(o, n):
            return SCR[:, o:o + n]

        def scr32(o, n):
            return SCR[:, o:o + 2 * n].bitcast(F32)

        def mk(flat, off, dims):
            return AP(tensor=flat.tensor, offset=flat.offset + off,
                      ap=[list(flat.ap[0])] + [list(d) for d in dims])

        ac16 = scr(AC16_O, 8 * PX).rearrange("p (c x) -> p c x", x=PX)
        astage = [scr32(AST_O + 2 * PX * i, PX) for i in range(4)]
        abstmp = scr(ABS_O, PX)
        sums = scr(SUMS_O, PX)
        suma = scr(SUMA_O, PX)
        rec16 = scr(REC16_O, PX)
        sums32 = scr32(AST_O + 2 * PX, PX)   # staging slot 1, dead by then
        rec32 = scr32(REC32_O, PX)
        scrr = scr32(AST_O, PX)              # staging slot 0, dead by then

        def f3(t):
            return t[:, :].rearrange("p (s w) -> p s w", w=WPAD)

        fbv = [f3(t) for t in fbp]
        fbsv = [f3(t) for t in fbsp]

        # ---- input loads: every transfer is split into partition-halves
        # placed on two of the three DMA queues (gps SWDGE + the two hwdge
        # rings) so all queues finish together; items ordered so channels
        # arrive in sum-chain order, feature (not in the chain) last ----
        feat_v = feat_d[:, :].rearrange("(p r) w -> p r w", r=ROWS_P)
        aff_v = aff_d[:, :].rearrange("c (p x) -> c p x", x=PX)
        CHAIN = [0, 1, 6, 2, 3, 7, 4, 5]
        RING = [3, 0, 1, 2, 3, 0, 1, 2, 3]   # staging slot per load item
        QA = [nc.gpsimd, nc.sync, nc.scalar]
        PH = P // 2

        def item_trigs(i):
            slot = RING[i]
            src = feat_v if i == 8 else aff_v[CHAIN[i]]
            dst = astage[slot]
            if i == 8:
                dst = dst.rearrange("p (r w) -> p r w", w=W)
            QA[(2 * i + 1) % 3].dma_start(dst[0:PH], src[0:PH])
            QA[(2 * i + 2) % 3].dma_start(dst[PH:P], src[PH:P])

        for i in range(3):
            item_trigs(i)

        # ---- zero-init padded state buffers (gpsimd; overlaps DMA) ----
        for t in fbp + fbsp:
            nc.gpsimd.memset(t[:, :], 0.0)

        # ---- fp16 convert + sum chain on DVE in channel-arrival order,
        # next item's triggers emitted as each staging slot frees ----
        for i, c in enumerate(CHAIN):
            st = astage[RING[i]]
            slab = SLAB_OF_CH[c]
            nc.vector.tensor_scalar(ac16[:, slab, :], st, 0.5, None, OP.mult)
            if i == 0:
                nc.vector.tensor_scalar(
                    sums.bitcast(I32), ac16[:, slab, :].bitcast(I32),
                    0x7FFF7FFF, None, OP.bitwise_and)
                nc.vector.tensor_copy(suma, ac16[:, slab, :])
            else:
                nc.vector.tensor_scalar(
                    abstmp.bitcast(I32), ac16[:, slab, :].bitcast(I32),
                    0x7FFF7FFF, None, OP.bitwise_and)
                nc.vector.tensor_add(sums, sums, abstmp)
                nc.vector.tensor_add(suma, suma, ac16[:, slab, :])
            if i + 3 <= 8:
                item_trigs(i + 3)

        # initial fp16 state: ACT is safe again here (all its queue
        # triggers are already emitted), overlapping the chain tail
        fst = astage[3].rearrange("p (r w) -> p r w", w=W)
        nc.scalar.activation(fbv[0][:, 1:5, 1:1 + W], fst, AF.Copy)
        nc.scalar.activation(fbsv[0][:, 1:5, 0:W], fst, AF.Copy)
        # initial halos on the gps queue (hwdge queues just drained)
        nc.gpsimd.dma_start(fbsv[0][1:P, 0, :], fbsv[0][0:P - 1, 4, :])
        nc.gpsimd.dma_start(fbsv[0][0:P - 1, 5, :], fbsv[0][1:P, 1, :])
        nc.gpsimd.dma_start(fbv[0][1:P, 0, :], fbv[0][0:P - 1, 4, :])
        nc.gpsimd.dma_start(fbv[0][0:P - 1, 5, :], fbv[0][1:P, 1, :])

        # ---- normalize: w9[0:8] = (a/2) * (1/S); center = 0.5 - suma/S ----
        nc.vector.tensor_copy(sums32, sums)
        nc.vector.reciprocal_approx_accurate(rec32, sums32, scrr)
        nc.vector.tensor_scalar(rec16, rec32, 0.5, None, OP.mult)
        # center weight on gpsimd, overlapping the big normalize mul on DVE
        ge = nc.gpsimd if GPS_TREE else nc.vector
        ge.tensor_mul(abstmp, suma, rec16)
        ge.tensor_scalar(w9[:, 8, :], abstmp, -1.0, 0.5, OP.mult, OP.add)
        nc.vector.tensor_mul(w9[:, 0:8, :], ac16[:, :, :],
                             rec16.unsqueeze(1).broadcast_to([P, 8, PX]))

        w9f = w9[:, :, :].rearrange("p c x -> p (c x)")
        out_v = out_d[:, :].rearrange("(p r) w -> p r w", r=ROWS_P)

        # ---- iterations ----
        for it in range(times):
            cb, nb = it % 2, (it + 1) % 2
            cfb, cfbs = fbp[cb][:, :], fbsp[cb][:, :]
            nfb3, nfbs3 = fbv[nb], fbsv[nb]
            nfbs = fbsp[nb][:, :]
            last = it == times - 1
            sc = SCB[it % 2]
            M4 = [[PX, 2], [W, ROWS_P], [1, W]]
            W4 = [[WPAD, ROWS_P], [1, W]]

            # muls, interior parts first (independent of halo DMAs so the
            # DVE never waits on the ~4.5us halo path at iter boundaries):
            # E (center) and D-interior read only DVE-written fbs rows.
            nc.vector.tensor_mul(
                scr(sc + 5 * PX, PX).rearrange("p (s w) -> p s w", w=W),
                mk(cfbs, WPAD, W4),
                w9[:, 8, :].rearrange("p (s w) -> p s w", w=W))
            nc.vector.tensor_mul(
                mk(scrf, sc + 3 * PX + W, [[3 * W, 2], [W, 3], [1, W]]),
                mk(cfbs, WPAD, [[WPAD, 2], [WPAD, 3], [1, W]]),
                mk(w9f, 6 * PX + W, [[3 * W, 2], [W, 3], [1, W]]))
            nc.vector.tensor_mul(mk(scrf, RR_O + 3 * PX, M4),
                                 mk(cfb, WPAD, [[2, 2]] + W4),
                                 mk(w9f, 2 * PX, M4))
            nc.vector.tensor_mul(mk(scrf, RR_O + W, [[PX, 2], [W, 3], [1, W]]),
                                 mk(cfb, WPAD, [[2, 2], [WPAD, 3], [1, W]]),
                                 mk(w9f, W, [[PX, 2], [W, 3], [1, W]]))
            nc.vector.tensor_mul(mk(scrf, RR_O + 2 * PX,
                                    [[3 * PX, 2], [W, 3], [1, W]]),
                                 mk(cfb, 2 * WPAD, [[2, 2], [WPAD, 3], [1, W]]),
                                 mk(w9f, 4 * PX, [[PX, 2], [W, 3], [1, W]]))
            # halo-row parts (1 row each; halos have landed by now)
            nc.vector.tensor_mul(
                mk(scrf, sc + 3 * PX, [[PX + 3 * W, 2], [1, W]]),
                mk(cfbs, 0, [[5 * WPAD, 2], [1, W]]),
                mk(w9f, 6 * PX, [[PX + 3 * W, 2], [1, W]]))
            nc.vector.tensor_mul(mk(scrf, RR_O, [[PX, 2], [1, W]]),
                                 mk(cfb, 0, [[2, 2], [1, W]]),
                                 mk(w9f, 0, [[PX, 2], [1, W]]))
            nc.vector.tensor_mul(mk(scrf, RR_O + 2 * PX + 3 * W,
                                    [[3 * PX, 2], [1, W]]),
                                 mk(cfb, 5 * WPAD, [[2, 2], [1, W]]),
                                 mk(w9f, 4 * PX + 3 * W, [[PX, 2], [1, W]]))
            # add tree over concatenated buffers
            nc.vector.tensor_add(scr(sc, 3 * PX), scr(RR_O, 3 * PX),
                                 scr(RR_O + 3 * PX, 3 * PX))
            l2 = mk(scrf, sc, [[2 * PX, 3], [1, PX]])
            nc.vector.tensor_tensor(l2, l2, mk(scrf, sc + PX,
                                               [[2 * PX, 3], [1, PX]]),
                                    OP.add)
            # L3/L4 split {rows 0,3} then {rows 1,2}; halos fire after L4a
            nc.vector.tensor_tensor(
                mk(scrf, sc, [[3 * W, 2], [1, W]]),
                mk(scrf, sc, [[3 * W, 2], [1, W]]),
                mk(scrf, sc + 2 * PX, [[3 * W, 2], [1, W]]), OP.add)
            nc.vector.tensor_tensor(
                mk(nfbs, WPAD, [[3 * WPAD, 2], [1, W]]),
                mk(scrf, sc, [[3 * W, 2], [1, W]]),
                mk(scrf, sc + 4 * PX, [[3 * W, 2], [1, W]]), OP.add)
            if not last:
                nc.sync.dma_start(nfbs3[1:P, 0, :], nfbs3[0:P - 1, 4, :])
                nc.scalar.dma_start(nfbs3[0:P - 1, 5, :], nfbs3[1:P, 1, :])
            nc.vector.tensor_tensor(
                mk(scrf, sc + W, [[W, 2], [1, W]]),
                mk(scrf, sc + W, [[W, 2], [1, W]]),
                mk(scrf, sc + 2 * PX + W, [[W, 2], [1, W]]), OP.add)
            nc.vector.tensor_tensor(
                mk(nfbs, 2 * WPAD, [[WPAD, 2], [1, W]]),
                mk(scrf, sc + W, [[W, 2], [1, W]]),
                mk(scrf, sc + 4 * PX + W, [[W, 2], [1, W]]), OP.add)
            if not last:
                # rebuild fb (shifted copy) + its halos
                nc.scalar.activation(mk(fbp[nb][:, :], WPAD + 1,
                                        [[3 * WPAD, 2], [1, W]]),
                                     mk(nfbs, WPAD, [[3 * WPAD, 2], [1, W]]),
                                     AF.Copy)
                nc.scalar.dma_start(nfb3[1:P, 0, :], nfb3[0:P - 1, 4, :])
                nc.sync.dma_start(nfb3[0:P - 1, 5, :], nfb3[1:P, 1, :])
                nc.scalar.activation(mk(fbp[nb][:, :], 2 * WPAD + 1,
                                        [[WPAD, 2], [1, W]]),
                                     mk(nfbs, 2 * WPAD, [[WPAD, 2], [1, W]]),
                                     AF.Copy)
            else:
                # unscale to fp32 and store, {rows 0,3} right after L4a
                o32 = scr32(REC32_O, PX).rearrange("p (r w) -> p r w", w=W)
                sc16 = float(2.0 ** times)
                nc.scalar.activation(o32[:, 0::3, :], nfbs3[:, 1::3, 0:W],
                                     AF.Copy, scale=sc16)
                nc.sync.dma_start(out_v[:, 0::3, :], o32[:, 0::3, :])
                nc.scalar.activation(o32[:, 1:3, :], nfbs3[:, 2:4, 0:W],
                                     AF.Copy, scale=sc16)
                nc.scalar.dma_start(out_v[:, 1:3, :], o32[:, 1:3, :])

        if times == 0:
            nc.sync.dma_start(
                out_v, astage[3].rearrange("p (r w) -> p r w", w=W))

    nc.compile()
    nc.m = get_hw_module(nc.m)
    return nc


_CACHE = {}


def _get(times: int):
    if times not in _CACHE:
        _CACHE[times] = _build(times)
    return _CACHE[times]


def kernel(affinity, feature, times, _trace=False, _trace_kwargs=None):
    t = int(times)
    nc = _get(t)
    aff = np.ascontiguousarray(affinity, dtype=np.float32)
    fea = np.ascontiguousarray(feature, dtype=np.float32)
    in_maps = [
        {"affinity": aff[b].reshape(CH, H * W), "feature": fea[b, 0]}
        for b in range(B)
    ]
    res = bass_utils.run_bass_kernel_spmd(
        nc, in_maps, core_ids=list(range(B)),
        trace=_trace, **(_trace_kwargs or {}),
    )
    out = np.stack([res.results[b]["out"] for b in range(B)])[:, None]
    if _trace:
        return out.astype(np.float32), res
    return out.astype(np.float32)


# revision 15
# speedup vs baseline: 1.2080x; 1.0069x over previous
"""AffinityPropagate prediction kernel for Trainium2 (8 NeuronCores).

Data-parallel over batch B=8: each core owns one image [480, 640].

Layout per core: 120 partitions x 4 image rows, rows padded to 642 cols.
All math runs in fp16 (DVE 2x mode). The center tap is folded into the
9-tap product tree with all weights pre-halved (w' = w/2), so the state
evolves as f * 2^-iter and stays in fp16 range with no per-iteration
rescale; one final ACT copy un-scales by 2^times.

State is kept in two aligned fp16 forms (double-buffered): fbs (image
cols at row offset 0, serving dc==0 taps) and fb (cols at offset 1,
serving dc==+-1 taps); every window is then 4B-aligned so every
tensor_tensor runs at DVE 2x. Per iteration DVE runs 11 instructions
(5 muls + 6 adds) with taps batched into multi-window access patterns
(tap-pair groups share one instruction) and the add tree batched over
concatenated product buffers. GPSIMD compute is deliberately unused in
the steady state: it shares an SBUF port with the DVE and measured 2.6x
slowdowns on concurrent tensor_tensor ops. The L3/final adds are split
{rows 0,3}/{rows 1,2} so halo-row DMAs fire one op early. DVE writes
fbs directly; ACT re-creates fb (shifted); SBUF-SBUF DMAs refresh
halos.
"""

import numpy as np
from contextlib import ExitStack

import concourse.bacc as bacc
import concourse.mybir as mybir
import concourse.tile as tile
from concourse import bass_utils
from concourse.ap import AP
from concourse.bass_interp import get_hw_module

B, CH, H, W = 8, 8, 480, 640
P = 120             # partitions (each holds ROWS_P rows)
ROWS_P = H // P     # 4
SLOTS = ROWS_P + 2  # + top/bottom halo row slots
WPAD = W + 2        # fb row: [0, img cols at 1..640, 0]
PX = ROWS_P * W     # 2560 compact px per partition

F32 = mybir.dt.float32
FP16 = mybir.dt.float16
I32 = mybir.dt.int32
AF = mybir.ActivationFunctionType
OP = mybir.AluOpType

# w9 slab order (taps grouped in instruction-pair order; center last):
#   0:(-1,-1) 1:(-1,+1) 2:(0,-1) 3:(0,+1) 4:(+1,-1) 5:(+1,+1)
#   6:(-1,0)  7:(+1,0)  8:center
# input affinity channel c (reference order) -> slab index:
SLAB_OF_CH = [0, 6, 1, 2, 3, 4, 7, 5]

# SCRATCH map (fp16 element offsets per partition)
AC16_O = 0            # [8, PX] fp16 copies of aff/2 in slab order (prep)
AST_O = 8 * PX        # 4-slot fp32 staging ring (2*PX units each)
SUMS_O = 16 * PX
SUMA_O = 17 * PX
REC16_O = 18 * PX
REC32_O = 19 * PX     # fp32 (2*PX units); also fp32 out staging at tail
ABS_O = 21 * PX       # fp16 abs scratch
SCR_UNITS = 22 * PX
RR_O = 0              # iter: two 3*PX-unit product rows
SCB = [6 * PX, 12 * PX]  # iter: X/D/E region, ring by parity

GPS_TREE = False      # gpsimd compute shares a DVE SBUF port: net loss (measured)


def _build(times: int):
    nc = bacc.Bacc("TRN2", debug=False, dynamic_dma_scratch_size=2048)
    aff_d = nc.dram_tensor("affinity", [CH, H * W], F32, kind="ExternalInput")
    feat_d = nc.dram_tensor("feature", [H, W], F32, kind="ExternalInput")
    out_d = nc.dram_tensor("out", [H, W], F32, kind="ExternalOutput")

    with tile.TileContext(nc) as tc, ExitStack() as ctx:
        pool = ctx.enter_context(tc.tile_pool(name="main", bufs=1))

        w9 = pool.tile([P, 9, PX], FP16)
        fbp = [pool.tile([P, SLOTS * WPAD], FP16, name=f"fb{i}")
               for i in range(2)]
        fbsp = [pool.tile([P, SLOTS * WPAD], FP16, name=f"fbs{i}")
                for i in range(2)]
        SCR = pool.tile([P, SCR_UNITS], FP16)

        scrf = SCR[:, :]

        def scr(o, n):
            return SCR[:, o:o + n]

        def scr32(o, n):
            return SCR[:, o:o + 2 * n].bitcast(F32)

        def mk(flat, off, dims):
            return AP(tensor=flat.tensor, offset=flat.offset + off,
                      ap=[list(flat.ap[0])] + [list(d) for d in dims])

        ac16 = scr(AC16_O, 8 * PX).rearrange("p (c x) -> p c x", x=PX)
        astage = [scr32(AST_O + 2 * PX * i, PX) for i in range(4)]
        abstmp = scr(ABS_O, PX)
        sums = scr(SUMS_O, PX)
        suma = scr(SUMA_O, PX)
        rec16 = scr(REC16_O, PX)
        sums32 = scr32(AST_O + 2 * PX, PX)   # staging slot 1, dead by then
        rec32 = scr32(REC32_O, PX)
        scrr = scr32(AST_O, PX)              # staging slot 0, dead by then

        def f3(t):
            return t[:, :].rearrange("p (s w) -> p s w", w=WPAD)

        fbv = [f3(t) for t in fbp]
        fbsv = [f3(t) for t in fbsp]

        # ---- input loads: every transfer is split into partition-halves
        # placed on two of the three DMA queues (gps SWDGE + the two hwdge
        # rings) so all queues finish together; items ordered so channels
        # arrive in sum-chain order, feature (not in the chain) last ----
        feat_v = feat_d[:, :].rearrange("(p r) w -> p r w", r=ROWS_P)
        aff_v = aff_d[:, :].rearrange("c (p x) -> c p x", x=PX)
        CHAIN = [0, 1, 6, 2, 3, 7, 4, 5]
        RING = [3, 0, 1, 2, 3, 0, 1, 2, 3]   # staging slot per load item
        QA = [nc.gpsimd, nc.sync, nc.scalar]
        PH = P // 2

        def item_trigs(i):
            slot = RING[i]
            src = feat_v if i == 8 else aff_v[CHAIN[i]]
            dst = astage[slot]
            if i == 8:
                dst = dst.rearrange("p (r w) -> p r w", w=W)
            QA[(2 * i + 2) % 3].dma_start(dst[0:PH], src[0:PH])
            QA[(2 * i) % 3].dma_start(dst[PH:P], src[PH:P])

        for i in range(3):
            item_trigs(i)

        # ---- zero-init padded state buffers (gpsimd; overlaps DMA) ----
        for t in fbp + fbsp:
            nc.gpsimd.memset(t[:, :], 0.0)

        # ---- fp16 convert + sum chain on DVE in channel-arrival order,
        # next item's triggers emitted as each staging slot frees ----
        for i, c in enumerate(CHAIN):
            st = astage[RING[i]]
            slab = SLAB_OF_CH[c]
            nc.vector.tensor_scalar(ac16[:, slab, :], st, 0.5, None, OP.mult)
            if i == 0:
                nc.vector.tensor_scalar(
                    sums.bitcast(I32), ac16[:, slab, :].bitcast(I32),
                    0x7FFF7FFF, None, OP.bitwise_and)
                nc.vector.tensor_copy(suma, ac16[:, slab, :])
            else:
                nc.vector.tensor_scalar(
                    abstmp.bitcast(I32), ac16[:, slab, :].bitcast(I32),
                    0x7FFF7FFF, None, OP.bitwise_and)
                nc.vector.tensor_add(sums, sums, abstmp)
                nc.vector.tensor_add(suma, suma, ac16[:, slab, :])
            if i + 3 <= 8:
                item_trigs(i + 3)

        # initial fp16 state: ACT is safe again here (all its queue
        # triggers are already emitted), overlapping the chain tail
        fst = astage[3].rearrange("p (r w) -> p r w", w=W)
        nc.scalar.activation(fbv[0][:, 1:5, 1:1 + W], fst, AF.Copy)
        nc.scalar.activation(fbsv[0][:, 1:5, 0:W], fst, AF.Copy)
        # initial halos on the gps queue (hwdge queues just drained)
        nc.gpsimd.dma_start(fbsv[0][1:P, 0, :], fbsv[0][0:P - 1, 4, :])
        nc.gpsimd.dma_start(fbsv[0][0:P - 1, 5, :], fbsv[0][1:P, 1, :])
        nc.gpsimd.dma_start(fbv[0][1:P, 0, :], fbv[0][0:P - 1, 4, :])
        nc.gpsimd.dma_start(fbv[0][0:P - 1, 5, :], fbv[0][1:P, 1, :])

        # ---- normalize: w9[0:8] = (a/2) * (1/S); center = 0.5 - suma/S ----
        nc.vector.tensor_copy(sums32, sums)
        nc.vector.reciprocal_approx_accurate(rec32, sums32, scrr)
        nc.vector.tensor_scalar(rec16, rec32, 0.5, None, OP.mult)
        # center weight on gpsimd, overlapping the big normalize mul on DVE
        ge = nc.gpsimd if GPS_TREE else nc.vector
        ge.tensor_mul(abstmp, suma, rec16)
        ge.tensor_scalar(w9[:, 8, :], abstmp, -1.0, 0.5, OP.mult, OP.add)
        nc.vector.tensor_mul(w9[:, 0:8, :], ac16[:, :, :],
                             rec16.unsqueeze(1).broadcast_to([P, 8, PX]))

        w9f = w9[:, :, :].rearrange("p c x -> p (c x)")
        out_v = out_d[:, :].rearrange("(p r) w -> p r w", r=ROWS_P)

        # ---- iterations ----
        for it in range(times):
            cb, nb = it % 2, (it + 1) % 2
            cfb, cfbs = fbp[cb][:, :], fbsp[cb][:, :]
            nfb3, nfbs3 = fbv[nb], fbsv[nb]
            nfbs = fbsp[nb][:, :]
            last = it == times - 1
            sc = SCB[it % 2]
            M4 = [[PX, 2], [W, ROWS_P], [1, W]]
            W4 = [[WPAD, ROWS_P], [1, W]]

            # muls, interior parts first (independent of halo DMAs so the
            # DVE never waits on the ~4.5us halo path at iter boundaries):
            # E (center) and D-interior read only DVE-written fbs rows.
            nc.vector.tensor_mul(
                scr(sc + 5 * PX, PX).rearrange("p (s w) -> p s w", w=W),
                mk(cfbs, WPAD, W4),
                w9[:, 8, :].rearrange("p (s w) -> p s w", w=W))
            nc.vector.tensor_mul(
                mk(scrf, sc + 3 * PX + W, [[3 * W, 2], [W, 3], [1, W]]),
                mk(cfbs, WPAD, [[WPAD, 2], [WPAD, 3], [1, W]]),
                mk(w9f, 6 * PX + W, [[3 * W, 2], [W, 3], [1, W]]))
            nc.vector.tensor_mul(mk(scrf, RR_O + 3 * PX, M4),
                                 mk(cfb, WPAD, [[2, 2]] + W4),
                                 mk(w9f, 2 * PX, M4))
            nc.vector.tensor_mul(mk(scrf, RR_O + W, [[PX, 2], [W, 3], [1, W]]),
                                 mk(cfb, WPAD, [[2, 2], [WPAD, 3], [1, W]]),
                                 mk(w9f, W, [[PX, 2], [W, 3], [1, W]]))
            nc.vector.tensor_mul(mk(scrf, RR_O + 2 * PX,
                                    [[3 * PX, 2], [W, 3], [1, W]]),
                                 mk(cfb, 2 * WPAD, [[2, 2], [WPAD, 3], [1, W]]),
                                 mk(w9f, 4 * PX, [[PX, 2], [W, 3], [1, W]]))
            # halo-row parts (1 row each; halos have landed by now)
            nc.vector.tensor_mul(
                mk(scrf, sc + 3 * PX, [[PX + 3 * W, 2], [1, W]]),
                mk(cfbs, 0, [[5 * WPAD, 2], [1, W]]),
                mk(w9f, 6 * PX, [[PX + 3 * W, 2], [1, W]]))
            nc.vector.tensor_mul(mk(scrf, RR_O, [[PX, 2], [1, W]]),
                                 mk(cfb, 0, [[2, 2], [1, W]]),
                                 mk(w9f, 0, [[PX, 2], [1, W]]))
            nc.vector.tensor_mul(mk(scrf, RR_O + 2 * PX + 3 * W,
                                    [[3 * PX, 2], [1, W]]),
                                 mk(cfb, 5 * WPAD, [[2, 2], [1, W]]),
                                 mk(w9f, 4 * PX + 3 * W, [[PX, 2], [1, W]]))
            # add tree over concatenated buffers
            nc.vector.tensor_add(scr(sc, 3 * PX), scr(RR_O, 3 * PX),
                                 scr(RR_O + 3 * PX, 3 * PX))
            l2 = mk(scrf, sc, [[2 * PX, 3], [1, PX]])
            nc.vector.tensor_tensor(l2, l2, mk(scrf, sc + PX,
                                               [[2 * PX, 3], [1, PX]]),
                                    OP.add)
            # L3/L4 split {rows 0,3} then {rows 1,2}; halos fire after L4a
            nc.vector.tensor_tensor(
                mk(scrf, sc, [[3 * W, 2], [1, W]]),
                mk(scrf, sc, [[3 * W, 2], [1, W]]),
                mk(scrf, sc + 2 * PX, [[3 * W, 2], [1, W]]), OP.add)
            nc.vector.tensor_tensor(
                mk(nfbs, WPAD, [[3 * WPAD, 2], [1, W]]),
                mk(scrf, sc, [[3 * W, 2], [1, W]]),
                mk(scrf, sc + 4 * PX, [[3 * W, 2], [1, W]]), OP.add)
            if not last:
                nc.sync.dma_start(nfbs3[1:P, 0, :], nfbs3[0:P - 1, 4, :])
                nc.scalar.dma_start(nfbs3[0:P - 1, 5, :], nfbs3[1:P, 1, :])
            nc.vector.tensor_tensor(
                mk(scrf, sc + W, [[W, 2], [1, W]]),
                mk(scrf, sc + W, [[W, 2], [1, W]]),
                mk(scrf, sc + 2 * PX + W, [[W, 2], [1, W]]), OP.add)
            nc.vector.tensor_tensor(
                mk(nfbs, 2 * WPAD, [[WPAD, 2], [1, W]]),
                mk(scrf, sc + W, [[W, 2], [1, W]]),
                mk(scrf, sc + 4 * PX + W, [[W, 2], [1, W]]), OP.add)
            if not last:
                # rebuild fb (shifted copy) + its halos
                nc.scalar.activation(mk(fbp[nb][:, :], WPAD + 1,
                                        [[3 * WPAD, 2], [1, W]]),
                                     mk(nfbs, WPAD, [[3 * WPAD, 2], [1, W]]),
                                     AF.Copy)
                nc.scalar.dma_start(nfb3[1:P, 0, :], nfb3[0:P - 1, 4, :])
                nc.sync.dma_start(nfb3[0:P - 1, 5, :], nfb3[1:P, 1, :])
                nc.scalar.activation(mk(fbp[nb][:, :], 2 * WPAD + 1,
                                        [[WPAD, 2], [1, W]]),
                                     mk(nfbs, 2 * WPAD, [[WPAD, 2], [1, W]]),
                                     AF.Copy)
            else:
                # unscale to fp32 and store, {rows 0,3} right after L4a
                o32 = scr32(REC32_O, PX).rearrange("p (r w) -> p r w", w=W)
                sc16 = float(2.0 ** times)
                nc.scalar.activation(o32[:, 0::3, :], nfbs3[:, 1::3, 0:W],
                                     AF.Copy, scale=sc16)
                nc.sync.dma_start(out_v[:, 0::3, :], o32[:, 0::3, :])
                nc.scalar.activation(o32[:, 1:3, :], nfbs3[:, 2:4, 0:W],
                                     AF.Copy, scale=sc16)
                nc.scalar.dma_start(out_v[:, 1:3, :], o32[:, 1:3, :])

        if times == 0:
            nc.sync.dma_start(
                out_v, astage[3].rearrange("p (r w) -> p r w", w=W))

    nc.compile()
    nc.m = get_hw_module(nc.m)
    return nc


_CACHE = {}


def _get(times: int):
    if times not in _CACHE:
        _CACHE[times] = _build(times)
    return _CACHE[times]


def kernel(affinity, feature, times, _trace=False, _trace_kwargs=None):
    t = int(times)
    nc = _get(t)
    aff = np.ascontiguousarray(affinity, dtype=np.float32)
    fea = np.ascontiguousarray(feature, dtype=np.float32)
    in_maps = [
        {"affinity": aff[b].reshape(CH, H * W), "feature": fea[b, 0]}
        for b in range(B)
    ]
    res = bass_utils.run_bass_kernel_spmd(
        nc, in_maps, core_ids=list(range(B)),
        trace=_trace, **(_trace_kwargs or {}),
    )
    out = np.stack([res.results[b]["out"] for b in range(B)])[:, None]
    if _trace:
        return out.astype(np.float32), res
    return out.astype(np.float32)
